# revision 32
# baseline (speedup 1.0000x reference)
"""Trainium2 Bass kernel for the Luong-attention layer (nn_AttentionLayer).

Math (reference):
    hs_proj = enc @ W_a.T + b_a                  # [S,B,H]
    scores[s,b] = hs_proj[s,b] . h_t[b]          # [S,B]
    scores += log(mask).T
    a = softmax(scores, axis=0)
    c_t[b] = sum_s a[s,b] * enc[s,b]             # [B,H]
    out = tanh([c_t, h_t] @ W_r.T + b_r)         # [B,H]

Restructuring used here:
  * scores[s,b] = enc[s,b] . u[b] with u = h_t @ W_a (b_a cancels in the
    softmax). u is a [B,H] vector batch — 17 MFLOP, 0.7% of the problem —
    computed on the host during input prep and shipped pre-replicated in
    the partition layout (urep[p] = u[p%8]), which removes the whole
    W_a/h_t device-side setup chain from the critical path.
  * softmax shift-invariance: subtract a fixed C=40 instead of the max.
  * Data-parallel over batch: 8 cores x 8 batches, no collectives.
    Each core streams its enc shard (32 MiB as fp16) from HBM exactly once.

Per-core device pipeline, SBUF partitions p = (s_sub 16, b 8), h on the
free axis; enc host-pre-permuted+fp16 so each 2 MiB tile is one contiguous
DMA. Score computation is batched on the DVE: custom op MUL_CUMSUM_ANT
computes cumsum(enc * urep) over 8 groups (4096 elems) per instruction;
group scores fall out as strided cumsum differences (DVE tensor_tensor,
166 ns per chunk). Per group: ACT spreads exp(score-C) into its b column
(Exp with per-partition bias, -C/-inf spread mask), PE accumulates
psum_ct += psp.T @ enc_group (bf16 x fp16). Tail: raw-ct transposes and
the output projection run before the softmax denominator is known; the
1/l scale is folded into the final (x*linv + oh) -> tanh step.
"""

import sys

if "/opt/trn_rl_repo" not in sys.path:
    sys.path.insert(0, "/opt/trn_rl_repo")

import numpy as np

import concourse.bacc as bacc
import concourse.mybir as mybir
from concourse import tile
from concourse.bass_utils import run_bass_kernel_spmd

S, B, H = 4096, 64, 512
NCORES = 8
BC = B // NCORES          # 8 batches per core
SS = 128 // BC            # 16 s-positions per group
S_TILE = 256              # s-positions per DMA tile
GPT = S_TILE // SS        # groups per DMA tile (16)
HT_G = GPT // 2           # groups per cumsum chunk (8)
HT_F = HT_G * H           # free elements per cumsum chunk (4096)
C_SHIFT = 40.0
NEG_INF = -1.0e30
F32 = mybir.dt.float32
F32R = mybir.dt.float32r
F16 = mybir.dt.float16
BF16 = mybir.dt.bfloat16
I32 = mybir.dt.int32
AF = mybir.ActivationFunctionType
ALU = mybir.AluOpType


def _register_mul_cumsum():
    """Register the custom DVE op out = cumsum(in0 * in1) (fp32 accumulate).

    Appended at the end of dve_ops.OPS (row 17; free_opcode_rows confirms
    it is unused). The uops sha is computed from lower() at registration
    time — same check as the pinned-sha workflow, just inline.
    """
    import concourse.dve_ops as dve_ops
    from concourse.dve_spec import Spec, Src0, Src1, scan, AluOp, lower, _has_src1
    from concourse.dve_uop import DveOpSpec

    for op in dve_ops.OPS:
        if op.name == "MUL_CUMSUM_ANT":
            return op

    spec = Spec(
        body=scan(AluOp.ADD, Src0 * Src1),
        reference=lambda in0, in1, s0, s1, imm2: np.cumsum(
            in0.astype(np.float32) * np.asarray(in1, dtype=np.float32),
            axis=-1, dtype=np.float32),
    )
    row = dve_ops._CUSTOM_DVE_ROW_BASE + len(dve_ops.OPS)
    shas = {}
    for ver in ("v3", "v4"):
        uops = lower(spec, ver=ver)
        shas[ver] = DveOpSpec(name="MUL_CUMSUM_ANT", opcode=row, uops=uops,
                              rd1_en=_has_src1(spec)).sha(ver)
    op = dve_ops.DveOp("MUL_CUMSUM_ANT", spec, subdim=False, uops_sha=shas)
    dve_ops.OPS.append(op)
    dve_ops._SUB_OPCODE_FOR_NAME[op.name] = row
    dve_ops.CUSTOM_DVE_SPECS[op.name] = op.spec
    return op


MUL_CUMSUM = _register_mul_cumsum()


def build_program(s_total=S, debug=False, enable_asserts=False,
                  enc_bufs=5, col_bufs=16, cum_bufs=3, ones_mask=True):
    nt = s_total // S_TILE        # DMA tiles
    ng = s_total // SS            # total groups

    nc = bacc.Bacc("TRN2", target_bir_lowering=False, debug=debug,
                   enable_asserts=enable_asserts, num_devices=NCORES)

    enc = nc.dram_tensor("enc", [nt, 128, GPT * H], F16, kind="ExternalInput").ap()
    urep_p = nc.dram_tensor("urep_p", [128, H], F16, kind="ExternalInput").ap()
    w_rT = nc.dram_tensor("w_rT", [2 * H, H], F16, kind="ExternalInput").ap()
    h_tT16 = nc.dram_tensor("h_tT16", [H, BC], F16, kind="ExternalInput").ap()
    mask_p = nc.dram_tensor("mask_p", [128, ng], I32, kind="ExternalInput").ap()
    b_r_rep = nc.dram_tensor("b_r_rep", [BC, H], F32, kind="ExternalInput").ap()
    r_t = nc.dram_tensor("r_t", [128, BC], F32, kind="ExternalInput").ap()
    m01_rep = nc.dram_tensor("m01_rep", [128, GPT * BC], F32, kind="ExternalInput").ap()
    idn = nc.dram_tensor("idn", [BC, BC], F32, kind="ExternalInput").ap()
    out = nc.dram_tensor("out", [BC, H], F32, kind="ExternalOutput").ap()

    with tile.TileContext(nc) as tc:
        with (
            tc.tile_pool(name="const", bufs=1) as cpool,
            tc.tile_pool(name="encp", bufs=enc_bufs) as encp,
            tc.tile_pool(name="etp", bufs=3) as etp,
            tc.tile_pool(name="pspp", bufs=3) as pspp,
            tc.tile_pool(name="cump", bufs=cum_bufs) as cump,
            tc.tile_pool(name="prodp", bufs=2) as prodp,
            tc.tile_pool(name="scrp", bufs=3) as scrp,
            tc.tile_pool(name="psum", bufs=1, space="PSUM") as pp,
            tc.tile_pool(name="psumtr", bufs=2, space="PSUM") as pptr,
        ):
            w_rT_sb = cpool.tile([128, 8 * H], F16)     # [128, (c8, n512)]
            h_tT16_sb = cpool.tile([128, 4 * BC], F16)
            mask_sb = cpool.tile([128, ng], I32)
            maskf_sb = cpool.tile([128, ng], F32)
            logm_sb = cpool.tile([128, ng], F32)
            urep_sb = cpool.tile([128, H], F16)
            urep_rep = cpool.tile([128, HT_F], F16)
            rT_sb = cpool.tile([128, BC], F32)
            pall_sb = cpool.tile([128, ng], F32)
            pall2_sb = cpool.tile([128, ng], F32)
            expv_sb = cpool.tile([128, ng], F32)
            rowsum_sb = cpool.tile([128, 1], F32)
            negc_sb = cpool.tile([128, 1], F32)
            escr_sb = cpool.tile([128, 1], F32)
            m01_sb = cpool.tile([128, GPT * BC], F32)
            idn_sb = cpool.tile([BC, BC], F32)
            brr_sb = cpool.tile([BC, H], F32)
            linv_sb = cpool.tile([BC, 1], F32)
            ctr_sb = cpool.tile([BC, H], F32)
            catT_sb = cpool.tile([128, 4 * BC], F16)
            out_sb = cpool.tile([BC, H], F32)
            o2_sb = cpool.tile([BC, H], F32)
            oh_sb = cpool.tile([BC, H], F32)

            # sync queue: urep first (small), then the enc stream.
            nc.sync.dma_start(urep_sb[:], urep_p[:])
            if not ones_mask:
                nc.scalar.dma_start(mask_sb[:], mask_p[:])
            nc.scalar.dma_start(m01_sb[:], m01_rep[:])
            nc.gpsimd.dma_start(
                h_tT16_sb[:].rearrange("p (c b) -> p c b", c=4),
                h_tT16.rearrange("(c p) b -> p c b", p=128))
            nc.gpsimd.dma_start(
                w_rT_sb[:].rearrange("p (c n) -> p c n", c=8),
                w_rT.rearrange("(c p) n -> p c n", p=128))
            nc.gpsimd.dma_start(rT_sb[:], r_t[:])
            nc.gpsimd.dma_start(idn_sb[:], idn[:])
            nc.gpsimd.dma_start(brr_sb[:], b_r_rep[:])

            nc.vector.memset(negc_sb[:], -C_SHIFT)
            # warm the Exp activation table while DMAs stream
            nc.scalar.activation(escr_sb[:], negc_sb[:], AF.Exp)

            for i in range(HT_G):
                nc.vector.tensor_copy(urep_rep[:, i * H:(i + 1) * H], urep_sb[:])

            # log-mask (general-mask path; all-ones mask -> zeros, skipped)
            if not ones_mask:
                nc.vector.tensor_copy(maskf_sb[:], mask_sb[:])
                nc.scalar.activation(logm_sb[:], maskf_sb[:], AF.Ln)

            # h_t half of the output projection only needs h_tT16/w_rT:
            # compute it during setup while PE is otherwise idle.
            psum_oh = pp.tile([BC, H], F32)
            for ic in range(4):
                nc.tensor.matmul(psum_oh[:],
                                 h_tT16_sb[:, ic * BC:(ic + 1) * BC],
                                 w_rT_sb[:, (ic + 4) * H:(ic + 5) * H],
                                 start=(ic == 0), stop=(ic == 3))
            nc.vector.tensor_add(oh_sb[:], psum_oh[:], brr_sb[:])

            psum_oc = pp.tile([BC, H], F32)
            psum_ct = pp.tile([BC, H], F32)
            psum_l = pp.tile([BC, 1], F32)

            def score_cumsum(enc_sb, fo, gflo, gcnt):
                """Group scores for gcnt groups starting at flat offset fo
                within enc_sb (global group gflo): fused multiply-cumsum on
                the DVE, then strided diffs (and +logm for general masks)."""
                nf = gcnt * H
                cums = cump.tile([128, 1 + HT_F], F32)
                nc.gpsimd.memset(cums[:, 0:1], 0.0)
                nc.vector._custom_dve(
                    MUL_CUMSUM, out=cums[:, 1:1 + nf],
                    in0=enc_sb[:, fo:fo + nf],
                    in1=urep_rep[:, 0:nf])
                cols = slice(gflo, gflo + gcnt)
                ends = cums[:, 1:1 + nf].rearrange(
                    "p (g h) -> p g h", h=H)[:, :, H - 1:H]
                starts = cums[:, 0:nf].rearrange(
                    "p (g h) -> p g h", h=H)[:, :, 0:1]
                if ones_mask:
                    nc.vector.tensor_tensor(pall2_sb[:, cols], ends,
                                            starts, ALU.subtract)
                else:
                    nc.vector.tensor_tensor(pall_sb[:, cols], ends,
                                            starts, ALU.subtract)
                    nc.vector.tensor_tensor(pall2_sb[:, cols],
                                            pall_sb[:, cols],
                                            logm_sb[:, cols], ALU.add)

            def score_act(enc_sb, fo, gflo, gcnt):
                """Offload path: fp16 products at DVE 2x rate, per-group
                reduction on the Scalar engine (Copy + accumulator)."""
                nf = gcnt * H
                prod = prodp.tile([128, HT_F], F16)
                nc.vector.tensor_tensor(prod[:, 0:nf], enc_sb[:, fo:fo + nf],
                                        urep_rep[:, 0:nf], ALU.mult)
                for g in range(gcnt):
                    gi = gflo + g
                    dst = pall2_sb if ones_mask else pall_sb
                    scr = scrp.tile([128, H], F16)
                    nc.scalar.activation(scr[:], prod[:, g * H:(g + 1) * H],
                                         AF.Copy,
                                         accum_out=dst[:, gi:gi + 1])
                if not ones_mask:
                    cols = slice(gflo, gflo + gcnt)
                    nc.vector.tensor_tensor(pall2_sb[:, cols],
                                            pall_sb[:, cols],
                                            logm_sb[:, cols], ALU.add)

            def consume(enc_sb, fo, gflo, gcnt):
                """One batched Exp (ACT) + one broadcast spread (GPSIMD)
                for gcnt groups, then their context matmuls."""
                cols = slice(gflo, gflo + gcnt)
                ev = etp.tile([128, GPT], F32)
                nc.scalar.activation(ev[:, 0:gcnt], pall2_sb[:, cols], AF.Exp,
                                     bias=negc_sb[:], scale=1.0)
                psp = pspp.tile([128, GPT * BC], BF16)
                nc.gpsimd.tensor_tensor(
                    psp[:, 0:gcnt * BC].rearrange("p (g b) -> p g b", b=BC),
                    m01_sb[:, 0:gcnt * BC].rearrange("p (g b) -> p g b", b=BC),
                    ev[:, 0:gcnt].unsqueeze(2).broadcast_to([128, gcnt, BC]),
                    ALU.mult)
                for g in range(gcnt):
                    gi = gflo + g
                    col = slice(fo + g * H, fo + (g + 1) * H)
                    nc.tensor.matmul(psum_ct[:], psp[:, g * BC:(g + 1) * BC],
                                     enc_sb[:, col],
                                     start=(gi == 0), stop=(gi == ng - 1))

            # Software-pipelined emission: the consume stage (Exp + spread +
            # context matmuls) of tile t-1 is emitted at the head of
            # iteration t, so each engine's in-order queue interleaves one
            # tile's scoring with the previous tile's consumption.
            pending = None
            for t in range(nt - 1):
                enc_sb = encp.tile([128, GPT * H], F16)
                if t == 0:
                    q_w = GPT * H // 4
                    for q in range(4):
                        nc.sync.dma_start(enc_sb[:, q * q_w:(q + 1) * q_w],
                                          enc[t, :, q * q_w:(q + 1) * q_w])
                else:
                    nc.sync.dma_start(enc_sb[:], enc[t])
                if pending is not None:
                    consume(pending, (t - 1) * GPT * 0, (t - 1) * GPT, GPT)
                g0 = t * GPT
                if t == 0:
                    # quarter-chunks: first cumsum starts after 0.5 MiB lands
                    qg = GPT // 4
                    for q in range(4):
                        score_cumsum(enc_sb, q * qg * H, g0 + q * qg, qg)
                else:
                    # steady tiles: second half ACT-offloaded (DVE products
                    # at 2x + Scalar reduces) emitted FIRST so the Scalar
                    # engine's reduce burst overlaps the half0 cumsum;
                    # then the half0 cumsum (DVE) and its diffs.
                    score_act(enc_sb, HT_F, g0 + HT_G, HT_G)
                    score_cumsum(enc_sb, 0, g0, HT_G)
                pending = enc_sb
            consume(pending, 0, (nt - 2) * GPT, GPT)
            # drain tile: quarter-chunks consumed inline for a short exit
            t = nt - 1
            enc_sb = encp.tile([128, GPT * H], F16)
            nc.sync.dma_start(enc_sb[:], enc[t])
            g0 = t * GPT
            qg = GPT // 4
            for q in range(4):
                score_cumsum(enc_sb, q * qg * H, g0 + q * qg, qg)
                consume(enc_sb, q * qg * H, g0 + q * qg, qg)

            # tail: the l-chain (big Exp + accum -> l -> 1/l) completes on
            # ACT/PE/DVE while the last chunk's psp exps and context matmuls
            # still run, so normalizing ct first costs no extra latency.
            nc.scalar.activation(expv_sb[:], pall2_sb[:], AF.Exp,
                                 bias=negc_sb[:], accum_out=rowsum_sb[:])
            nc.tensor.matmul(psum_l[:], rT_sb[:], rowsum_sb[:],
                             start=True, stop=True)
            nc.vector.reciprocal(linv_sb[:], psum_l[:])
            nc.vector.tensor_scalar_mul(ctr_sb[:], psum_ct[:], linv_sb[:])
            for hc in range(4):
                ptr = pptr.tile([128, BC], F32)
                nc.tensor.transpose(ptr[:], ctr_sb[:, hc * 128:(hc + 1) * 128],
                                    idn_sb[:])
                nc.scalar.copy(catT_sb[:, hc * BC:(hc + 1) * BC], ptr[:])
            for ic in range(4):
                nc.tensor.matmul(psum_oc[:], catT_sb[:, ic * BC:(ic + 1) * BC],
                                 w_rT_sb[:, ic * H:(ic + 1) * H],
                                 start=(ic == 0), stop=(ic == 3))
            nc.vector.tensor_add(o2_sb[:], psum_oc[:], oh_sb[:])
            nc.scalar.activation(out_sb[:], o2_sb[:], AF.Tanh)
            nc.sync.dma_start(out[:], out_sb[:])

    nc.compile()
    return nc


def prep_in_maps(inputs, s_total=S):
    enc = np.asarray(inputs["encoder_hidden_states"]).astype(np.float32, copy=False)
    h_t = np.asarray(inputs["h_t"]).astype(np.float32, copy=False)
    mask = np.asarray(inputs["encoder_context_mask"]).astype(np.int32, copy=False)
    W_a = np.ascontiguousarray(np.asarray(inputs["W_a"], dtype=np.float32))
    W_r = np.asarray(inputs["W_r"]).astype(np.float32, copy=False)
    b_r = np.asarray(inputs["b_r"]).astype(np.float32, copy=False)

    ng = s_total // SS
    w_rT = np.ascontiguousarray(W_r.T.astype(np.float16))
    p_idx = np.arange(128)
    b_idx = np.arange(BC)
    r_mat = (p_idx[None, :] % BC == b_idx[:, None]).astype(np.float32)
    r_t = np.ascontiguousarray(r_mat.T)
    m01_rep = np.ascontiguousarray(np.tile(
        (p_idx[:, None] % BC == b_idx[None, :]).astype(np.float32), (1, GPT)))
    idn = np.eye(BC, dtype=np.float32)
    b_r_rep = np.ascontiguousarray(np.broadcast_to(b_r, (BC, H)))
    u_full = h_t @ W_a                # [B, H]; score[s,b] = enc[s,b] . u[b]

    in_maps = []
    for c in range(NCORES):
        bs = slice(c * BC, (c + 1) * BC)
        mask_c = mask[bs, :s_total]
        mask_p = np.ascontiguousarray(
            mask_c.reshape(BC, ng, SS).transpose(2, 0, 1).reshape(128, ng))
        urep_p = np.ascontiguousarray(
            u_full[bs][p_idx % BC, :].astype(np.float16))
        in_maps.append({
            "enc": np.ascontiguousarray(
                enc[:s_total, bs, :]
                .reshape(s_total // S_TILE, S_TILE // SS, SS, BC, H)
                .transpose(0, 2, 3, 1, 4)
                .reshape(s_total // S_TILE, 128, (S_TILE // SS) * H)
                .astype(np.float16)),
            "urep_p": urep_p,
            "h_tT16": np.ascontiguousarray(h_t[bs].T.astype(np.float16)),
            "w_rT": w_rT,
            "mask_p": mask_p,
            "b_r_rep": b_r_rep,
            "r_t": r_t,
            "m01_rep": m01_rep,
            "idn": idn,
        })
    return in_maps


_CACHE = {}


def _reset_device():
    # Best-effort recovery of a wedged NeuronCore left by a previous process.
    try:
        import ctypes
        lib = ctypes.CDLL("/opt/axon/libaxon_pjrt.so")
        lib.axon_reset.restype = ctypes.c_int64
        import jax
        jax.devices()
        lib.axon_reset()
    except Exception:
        pass


def run(inputs, trace=False, **kw):
    ones = bool(np.all(np.asarray(inputs["encoder_context_mask"]) == 1))
    key = ("nc", ones)
    if key not in _CACHE:
        _CACHE[key] = build_program(ones_mask=ones)
    nc = _CACHE[key]
    in_maps = prep_in_maps(inputs)
    try:
        res = run_bass_kernel_spmd(nc, in_maps, list(range(NCORES)),
                                   trace=trace, **kw)
    except Exception:
        _reset_device()
        res = run_bass_kernel_spmd(nc, in_maps, list(range(NCORES)),
                                   trace=trace, **kw)
    full = np.concatenate([np.asarray(res.results[c]["out"])
                           for c in range(NCORES)], axis=0).astype(np.float32)
    return full, res


def kernel(**inputs):
    return run(inputs)[0]


# revision 33
# speedup vs baseline: 1.1977x; 1.1977x over previous
"""Trainium2 Bass kernel for the Luong-attention layer (nn_AttentionLayer).

Math (reference):
    hs_proj = enc @ W_a.T + b_a                  # [S,B,H]
    scores[s,b] = hs_proj[s,b] . h_t[b]          # [S,B]
    scores += log(mask).T
    a = softmax(scores, axis=0)
    c_t[b] = sum_s a[s,b] * enc[s,b]             # [B,H]
    out = tanh([c_t, h_t] @ W_r.T + b_r)         # [B,H]

Restructuring used here:
  * scores[s,b] = enc[s,b] . u[b] with u = h_t @ W_a (b_a cancels in the
    softmax). u is a [B,H] vector batch — 17 MFLOP, 0.7% of the problem —
    computed on the host during input prep and shipped pre-replicated in
    the partition layout (urep[p] = u[p%8]), which removes the whole
    W_a/h_t device-side setup chain from the critical path.
  * softmax shift-invariance: subtract a fixed C=40 instead of the max.
  * Data-parallel over batch: 8 cores x 8 batches, no collectives.
    Each core streams its enc shard (32 MiB as fp16) from HBM exactly once.

Per-core device pipeline, SBUF partitions p = (s_sub 16, b 8), h on the
free axis; enc host-pre-permuted+fp16 so each 2 MiB tile is one contiguous
DMA. Score computation is batched on the DVE: custom op MUL_CUMSUM_ANT
computes cumsum(enc * urep) over 8 groups (4096 elems) per instruction;
group scores fall out as strided cumsum differences (DVE tensor_tensor,
166 ns per chunk). Per group: ACT spreads exp(score-C) into its b column
(Exp with per-partition bias, -C/-inf spread mask), PE accumulates
psum_ct += psp.T @ enc_group (bf16 x fp16). Tail: raw-ct transposes and
the output projection run before the softmax denominator is known; the
1/l scale is folded into the final (x*linv + oh) -> tanh step.
"""

import sys

if "/opt/trn_rl_repo" not in sys.path:
    sys.path.insert(0, "/opt/trn_rl_repo")

import numpy as np

import concourse.bacc as bacc
import concourse.mybir as mybir
from concourse import tile
from concourse.bass_utils import run_bass_kernel_spmd

S, B, H = 4096, 64, 512
NCORES = 8
BC = B // NCORES          # 8 batches per core
SS = 128 // BC            # 16 s-positions per group
S_TILE = 256              # s-positions per DMA tile
GPT = S_TILE // SS        # groups per DMA tile (16)
HT_G = GPT // 2           # groups per cumsum chunk (8)
HT_F = HT_G * H           # free elements per cumsum chunk (4096)
C_SHIFT = 40.0
NEG_INF = -1.0e30
F32 = mybir.dt.float32
F32R = mybir.dt.float32r
F16 = mybir.dt.float16
BF16 = mybir.dt.bfloat16
I32 = mybir.dt.int32
AF = mybir.ActivationFunctionType
ALU = mybir.AluOpType


def _register_mul_cumsum():
    """Register the custom DVE op out = cumsum(in0 * in1) (fp32 accumulate).

    Appended at the end of dve_ops.OPS (row 17; free_opcode_rows confirms
    it is unused). The uops sha is computed from lower() at registration
    time — same check as the pinned-sha workflow, just inline.
    """
    import concourse.dve_ops as dve_ops
    from concourse.dve_spec import Spec, Src0, Src1, scan, AluOp, lower, _has_src1
    from concourse.dve_uop import DveOpSpec

    for op in dve_ops.OPS:
        if op.name == "MUL_CUMSUM_ANT":
            return op

    spec = Spec(
        body=scan(AluOp.ADD, Src0 * Src1),
        reference=lambda in0, in1, s0, s1, imm2: np.cumsum(
            in0.astype(np.float32) * np.asarray(in1, dtype=np.float32),
            axis=-1, dtype=np.float32),
    )
    row = dve_ops._CUSTOM_DVE_ROW_BASE + len(dve_ops.OPS)
    shas = {}
    for ver in ("v3", "v4"):
        uops = lower(spec, ver=ver)
        shas[ver] = DveOpSpec(name="MUL_CUMSUM_ANT", opcode=row, uops=uops,
                              rd1_en=_has_src1(spec)).sha(ver)
    op = dve_ops.DveOp("MUL_CUMSUM_ANT", spec, subdim=False, uops_sha=shas)
    dve_ops.OPS.append(op)
    dve_ops._SUB_OPCODE_FOR_NAME[op.name] = row
    dve_ops.CUSTOM_DVE_SPECS[op.name] = op.spec
    return op


MUL_CUMSUM = _register_mul_cumsum()


def build_program(s_total=S, debug=False, enable_asserts=False,
                  enc_bufs=5, col_bufs=16, cum_bufs=3, ones_mask=True):
    nt = s_total // S_TILE        # DMA tiles
    ng = s_total // SS            # total groups

    nc = bacc.Bacc("TRN2", target_bir_lowering=False, debug=debug,
                   enable_asserts=enable_asserts, num_devices=NCORES)

    enc = nc.dram_tensor("enc", [nt, 128, GPT * H], F16, kind="ExternalInput").ap()
    urep_p = nc.dram_tensor("urep_p", [128, H], F16, kind="ExternalInput").ap()
    w_rT = nc.dram_tensor("w_rT", [2 * H, H], F16, kind="ExternalInput").ap()
    h_tT16 = nc.dram_tensor("h_tT16", [H, BC], F16, kind="ExternalInput").ap()
    mask_p = nc.dram_tensor("mask_p", [128, ng], I32, kind="ExternalInput").ap()
    b_r_rep = nc.dram_tensor("b_r_rep", [BC, H], F32, kind="ExternalInput").ap()
    r_t = nc.dram_tensor("r_t", [128, BC], F32, kind="ExternalInput").ap()
    m01_rep = nc.dram_tensor("m01_rep", [128, GPT * BC], F32, kind="ExternalInput").ap()
    idn = nc.dram_tensor("idn", [BC, BC], F32, kind="ExternalInput").ap()
    out = nc.dram_tensor("out", [BC, H], F32, kind="ExternalOutput").ap()

    with tile.TileContext(nc) as tc:
        with (
            tc.tile_pool(name="const", bufs=1) as cpool,
            tc.tile_pool(name="encp", bufs=enc_bufs) as encp,
            tc.tile_pool(name="etp", bufs=3) as etp,
            tc.tile_pool(name="pspp", bufs=3) as pspp,
            tc.tile_pool(name="cump", bufs=cum_bufs) as cump,
            tc.tile_pool(name="prodp", bufs=2) as prodp,
            tc.tile_pool(name="scrp", bufs=3) as scrp,
            tc.tile_pool(name="psum", bufs=1, space="PSUM") as pp,
            tc.tile_pool(name="psumtr", bufs=2, space="PSUM") as pptr,
        ):
            w_rT_sb = cpool.tile([128, 8 * H], F16)     # [128, (c8, n512)]
            h_tT16_sb = cpool.tile([128, 4 * BC], F16)
            mask_sb = cpool.tile([128, ng], I32)
            maskf_sb = cpool.tile([128, ng], F32)
            logm_sb = cpool.tile([128, ng], F32)
            urep_sb = cpool.tile([128, H], F16)
            urep_rep = cpool.tile([128, HT_F], F16)
            rT_sb = cpool.tile([128, BC], F32)
            pall_sb = cpool.tile([128, ng], F32)
            pall2_sb = cpool.tile([128, ng], F32)
            expv_sb = cpool.tile([128, ng], F32)
            rowsum_sb = cpool.tile([128, 1], F32)
            negc_sb = cpool.tile([128, 1], F32)
            escr_sb = cpool.tile([128, 1], F32)
            m01_sb = cpool.tile([128, GPT * BC], F32)
            idn_sb = cpool.tile([BC, BC], F32)
            brr_sb = cpool.tile([BC, H], F32)
            linv_sb = cpool.tile([BC, 1], F32)
            ctr_sb = cpool.tile([BC, H], F32)
            catT_sb = cpool.tile([128, 4 * BC], F16)
            out_sb = cpool.tile([BC, H], F32)
            o2_sb = cpool.tile([BC, H], F32)
            oh_sb = cpool.tile([BC, H], F32)

            # sync queue: urep first (small), then the enc stream.
            nc.sync.dma_start(urep_sb[:], urep_p[:])
            if not ones_mask:
                nc.scalar.dma_start(mask_sb[:], mask_p[:])
            nc.scalar.dma_start(m01_sb[:], m01_rep[:])
            nc.gpsimd.dma_start(
                h_tT16_sb[:].rearrange("p (c b) -> p c b", c=4),
                h_tT16.rearrange("(c p) b -> p c b", p=128))
            nc.gpsimd.dma_start(
                w_rT_sb[:].rearrange("p (c n) -> p c n", c=8),
                w_rT.rearrange("(c p) n -> p c n", p=128))
            nc.gpsimd.dma_start(rT_sb[:], r_t[:])
            nc.gpsimd.dma_start(idn_sb[:], idn[:])
            nc.gpsimd.dma_start(brr_sb[:], b_r_rep[:])

            nc.vector.memset(negc_sb[:], -C_SHIFT)
            # warm the Exp activation table while DMAs stream
            nc.scalar.activation(escr_sb[:], negc_sb[:], AF.Exp)

            for i in range(HT_G):
                nc.vector.tensor_copy(urep_rep[:, i * H:(i + 1) * H], urep_sb[:])

            # log-mask (general-mask path; all-ones mask -> zeros, skipped)
            if not ones_mask:
                nc.vector.tensor_copy(maskf_sb[:], mask_sb[:])
                nc.scalar.activation(logm_sb[:], maskf_sb[:], AF.Ln)

            # h_t half of the output projection only needs h_tT16/w_rT:
            # compute it during setup while PE is otherwise idle.
            psum_oh = pp.tile([BC, H], F32)
            for ic in range(4):
                nc.tensor.matmul(psum_oh[:],
                                 h_tT16_sb[:, ic * BC:(ic + 1) * BC],
                                 w_rT_sb[:, (ic + 4) * H:(ic + 5) * H],
                                 start=(ic == 0), stop=(ic == 3))
            nc.vector.tensor_add(oh_sb[:], psum_oh[:], brr_sb[:])

            psum_oc = pp.tile([BC, H], F32)
            psum_ct = pp.tile([BC, H], F32)
            psum_l = pp.tile([BC, 1], F32)

            def score_cumsum(enc_sb, fo, gflo, gcnt):
                """Group scores for gcnt groups starting at flat offset fo
                within enc_sb (global group gflo): fused multiply-cumsum on
                the DVE, then strided diffs (and +logm for general masks)."""
                nf = gcnt * H
                cums = cump.tile([128, 1 + HT_F], F32)
                nc.gpsimd.memset(cums[:, 0:1], 0.0)
                nc.vector._custom_dve(
                    MUL_CUMSUM, out=cums[:, 1:1 + nf],
                    in0=enc_sb[:, fo:fo + nf],
                    in1=urep_rep[:, 0:nf])
                cols = slice(gflo, gflo + gcnt)
                ends = cums[:, 1:1 + nf].rearrange(
                    "p (g h) -> p g h", h=H)[:, :, H - 1:H]
                starts = cums[:, 0:nf].rearrange(
                    "p (g h) -> p g h", h=H)[:, :, 0:1]
                if ones_mask:
                    nc.vector.tensor_tensor(pall2_sb[:, cols], ends,
                                            starts, ALU.subtract)
                else:
                    nc.vector.tensor_tensor(pall_sb[:, cols], ends,
                                            starts, ALU.subtract)
                    nc.vector.tensor_tensor(pall2_sb[:, cols],
                                            pall_sb[:, cols],
                                            logm_sb[:, cols], ALU.add)

            def score_act(enc_sb, fo, gflo, gcnt):
                """Offload path: fp16 products at DVE 2x rate, per-group
                reduction on the Scalar engine (Copy + accumulator)."""
                nf = gcnt * H
                prod = prodp.tile([128, HT_F], F16)
                nc.vector.tensor_tensor(prod[:, 0:nf], enc_sb[:, fo:fo + nf],
                                        urep_rep[:, 0:nf], ALU.mult)
                for g in range(gcnt):
                    gi = gflo + g
                    dst = pall2_sb if ones_mask else pall_sb
                    scr = scrp.tile([128, H], F16)
                    nc.scalar.activation(scr[:], prod[:, g * H:(g + 1) * H],
                                         AF.Copy,
                                         accum_out=dst[:, gi:gi + 1])
                if not ones_mask:
                    cols = slice(gflo, gflo + gcnt)
                    nc.vector.tensor_tensor(pall2_sb[:, cols],
                                            pall_sb[:, cols],
                                            logm_sb[:, cols], ALU.add)

            def consume(enc_sb, fo, gflo, gcnt):
                """One batched Exp (ACT) + one broadcast spread (GPSIMD)
                for gcnt groups, then their context matmuls."""
                cols = slice(gflo, gflo + gcnt)
                ev = etp.tile([128, GPT], F32)
                nc.scalar.activation(ev[:, 0:gcnt], pall2_sb[:, cols], AF.Exp,
                                     bias=negc_sb[:], scale=1.0)
                psp = pspp.tile([128, GPT * BC], BF16)
                nc.vector.tensor_tensor(
                    psp[:, 0:gcnt * BC].rearrange("p (g b) -> p g b", b=BC),
                    m01_sb[:, 0:gcnt * BC].rearrange("p (g b) -> p g b", b=BC),
                    ev[:, 0:gcnt].unsqueeze(2).broadcast_to([128, gcnt, BC]),
                    ALU.mult)
                for g in range(gcnt):
                    gi = gflo + g
                    col = slice(fo + g * H, fo + (g + 1) * H)
                    nc.tensor.matmul(psum_ct[:], psp[:, g * BC:(g + 1) * BC],
                                     enc_sb[:, col],
                                     start=(gi == 0), stop=(gi == ng - 1))

            # Software-pipelined emission: the consume stage (Exp + spread +
            # context matmuls) of tile t-1 is emitted at the head of
            # iteration t, so each engine's in-order queue interleaves one
            # tile's scoring with the previous tile's consumption.
            pending = None
            for t in range(nt - 1):
                enc_sb = encp.tile([128, GPT * H], F16)
                if t == 0:
                    q_w = GPT * H // 4
                    for q in range(4):
                        nc.sync.dma_start(enc_sb[:, q * q_w:(q + 1) * q_w],
                                          enc[t, :, q * q_w:(q + 1) * q_w])
                else:
                    nc.sync.dma_start(enc_sb[:], enc[t])
                if pending is not None:
                    consume(pending, (t - 1) * GPT * 0, (t - 1) * GPT, GPT)
                g0 = t * GPT
                if t == 0:
                    # quarter-chunks: first cumsum starts after 0.5 MiB lands
                    qg = GPT // 4
                    for q in range(4):
                        score_cumsum(enc_sb, q * qg * H, g0 + q * qg, qg)
                else:
                    # steady tiles: second half ACT-offloaded (DVE products
                    # at 2x + Scalar reduces) emitted FIRST so the Scalar
                    # engine's reduce burst overlaps the half0 cumsum;
                    # then the half0 cumsum (DVE) and its diffs.
                    score_act(enc_sb, HT_F, g0 + HT_G, HT_G)
                    score_cumsum(enc_sb, 0, g0, HT_G)
                pending = enc_sb
            consume(pending, 0, (nt - 2) * GPT, GPT)
            # drain tile: quarter-chunks consumed inline for a short exit
            t = nt - 1
            enc_sb = encp.tile([128, GPT * H], F16)
            nc.sync.dma_start(enc_sb[:], enc[t])
            g0 = t * GPT
            qg = GPT // 4
            for q in range(4):
                score_cumsum(enc_sb, q * qg * H, g0 + q * qg, qg)
                consume(enc_sb, q * qg * H, g0 + q * qg, qg)

            # tail: the l-chain (big Exp + accum -> l -> 1/l) completes on
            # ACT/PE/DVE while the last chunk's psp exps and context matmuls
            # still run, so normalizing ct first costs no extra latency.
            nc.scalar.activation(expv_sb[:], pall2_sb[:], AF.Exp,
                                 bias=negc_sb[:], accum_out=rowsum_sb[:])
            nc.tensor.matmul(psum_l[:], rT_sb[:], rowsum_sb[:],
                             start=True, stop=True)
            nc.vector.reciprocal(linv_sb[:], psum_l[:])
            nc.vector.tensor_scalar_mul(ctr_sb[:], psum_ct[:], linv_sb[:])
            for hc in range(4):
                ptr = pptr.tile([128, BC], F32)
                nc.tensor.transpose(ptr[:], ctr_sb[:, hc * 128:(hc + 1) * 128],
                                    idn_sb[:])
                nc.scalar.copy(catT_sb[:, hc * BC:(hc + 1) * BC], ptr[:])
            for ic in range(4):
                nc.tensor.matmul(psum_oc[:], catT_sb[:, ic * BC:(ic + 1) * BC],
                                 w_rT_sb[:, ic * H:(ic + 1) * H],
                                 start=(ic == 0), stop=(ic == 3))
            nc.vector.tensor_add(o2_sb[:], psum_oc[:], oh_sb[:])
            nc.scalar.activation(out_sb[:], o2_sb[:], AF.Tanh)
            nc.sync.dma_start(out[:], out_sb[:])

    nc.compile()
    return nc


def prep_in_maps(inputs, s_total=S):
    enc = np.asarray(inputs["encoder_hidden_states"]).astype(np.float32, copy=False)
    h_t = np.asarray(inputs["h_t"]).astype(np.float32, copy=False)
    mask = np.asarray(inputs["encoder_context_mask"]).astype(np.int32, copy=False)
    W_a = np.ascontiguousarray(np.asarray(inputs["W_a"], dtype=np.float32))
    W_r = np.asarray(inputs["W_r"]).astype(np.float32, copy=False)
    b_r = np.asarray(inputs["b_r"]).astype(np.float32, copy=False)

    ng = s_total // SS
    w_rT = np.ascontiguousarray(W_r.T.astype(np.float16))
    p_idx = np.arange(128)
    b_idx = np.arange(BC)
    r_mat = (p_idx[None, :] % BC == b_idx[:, None]).astype(np.float32)
    r_t = np.ascontiguousarray(r_mat.T)
    m01_rep = np.ascontiguousarray(np.tile(
        (p_idx[:, None] % BC == b_idx[None, :]).astype(np.float32), (1, GPT)))
    idn = np.eye(BC, dtype=np.float32)
    b_r_rep = np.ascontiguousarray(np.broadcast_to(b_r, (BC, H)))
    u_full = h_t @ W_a                # [B, H]; score[s,b] = enc[s,b] . u[b]

    in_maps = []
    for c in range(NCORES):
        bs = slice(c * BC, (c + 1) * BC)
        mask_c = mask[bs, :s_total]
        mask_p = np.ascontiguousarray(
            mask_c.reshape(BC, ng, SS).transpose(2, 0, 1).reshape(128, ng))
        urep_p = np.ascontiguousarray(
            u_full[bs][p_idx % BC, :].astype(np.float16))
        in_maps.append({
            "enc": np.ascontiguousarray(
                enc[:s_total, bs, :]
                .reshape(s_total // S_TILE, S_TILE // SS, SS, BC, H)
                .transpose(0, 2, 3, 1, 4)
                .reshape(s_total // S_TILE, 128, (S_TILE // SS) * H)
                .astype(np.float16)),
            "urep_p": urep_p,
            "h_tT16": np.ascontiguousarray(h_t[bs].T.astype(np.float16)),
            "w_rT": w_rT,
            "mask_p": mask_p,
            "b_r_rep": b_r_rep,
            "r_t": r_t,
            "m01_rep": m01_rep,
            "idn": idn,
        })
    return in_maps


_CACHE = {}


def _reset_device():
    # Best-effort recovery of a wedged NeuronCore left by a previous process.
    try:
        import ctypes
        lib = ctypes.CDLL("/opt/axon/libaxon_pjrt.so")
        lib.axon_reset.restype = ctypes.c_int64
        import jax
        jax.devices()
        lib.axon_reset()
    except Exception:
        pass


def run(inputs, trace=False, **kw):
    ones = bool(np.all(np.asarray(inputs["encoder_context_mask"]) == 1))
    key = ("nc", ones)
    if key not in _CACHE:
        _CACHE[key] = build_program(ones_mask=ones)
    nc = _CACHE[key]
    in_maps = prep_in_maps(inputs)
    try:
        res = run_bass_kernel_spmd(nc, in_maps, list(range(NCORES)),
                                   trace=trace, **kw)
    except Exception:
        _reset_device()
        res = run_bass_kernel_spmd(nc, in_maps, list(range(NCORES)),
                                   trace=trace, **kw)
    full = np.concatenate([np.asarray(res.results[c]["out"])
                           for c in range(NCORES)], axis=0).astype(np.float32)
    return full, res


def kernel(**inputs):
    return run(inputs)[0]
